# revision 28
# baseline (speedup 1.0000x reference)
"""MoE dense all-experts (GPT-OSS Experts forward) on 8 Trainium2 NeuronCores.

Expert-parallel sharding: core e holds expert e's weights and computes its
weighted contribution

    partial_e[t, h] = w[t, e] * ((up + 1) * silu(1.702 * gate) @ down_e.T)

with [gate | up] = hs @ gup_e + bias (the host de-interleaves gup's even/odd
columns so gate/up become contiguous halves). Each core writes its full
[T, H] partial to DRAM; the host sums the 8 partials and adds the
routing-weighted down-bias term (routing_weights @ down_bias) in fp32.

All matmul operands are float16 (fp32 PSUM accumulation): fp32/fp32r moving
operands stream through the PE at ~0.56 ns/col while 16-bit operands stream
at 1 col/cycle @ 2.4 GHz (0.417 ns/col), a 1.35x speedup at identical MAC
count; fp16's 10-bit mantissa keeps the end-to-end relative error ~5e-4.
Weights are staged in (j, kc)-block order so the gate pass starts after
~1.25 MB of DMA instead of waiting for the full weight tensor, and 16
dependency-free warm-up matmuls keep the PE busy through the DMA preamble
so the HAM clock gate releases (1.2 -> 2.4 GHz) before real work starts.

Stage 1 computes [f, t] tiles (gate pass feeding the ScalarE Silu LUT, then
up pass fused with the silu output via scalar_tensor_tensor into fp16
act[i, t]); stage 2 computes out[t, h] = (act.T @ dwT) * w[t] with act as
the stationary operand and a single VectorE tensor_scalar epilogue.
"""
import sys
if '/opt/trn_rl_repo' not in sys.path:
    sys.path.insert(0, '/opt/trn_rl_repo')
import numpy as np

E, H, I, T = 8, 1024, 1024, 4096
N_CORES = 8
TC = 512               # token chunk
CHUNKS = [TC] * 8
KC = H // 128          # contraction chunks (H == I == 1024)
NJ = I // 128          # gate/up row tiles

_CACHE = {}


def _build():
    import concourse.bacc as bacc
    import concourse.tile as tile
    import concourse.mybir as mybir
    f32 = mybir.dt.float32
    f16 = mybir.dt.float16
    AF = mybir.ActivationFunctionType
    ALU = mybir.AluOpType

    nc = bacc.Bacc("TRN2", target_bir_lowering=False, debug=False,
                   enable_asserts=False, num_devices=N_CORES)
    # hsR is host-pre-arranged into the exact SBUF chunk layout
    # hsR[p, (c*KC + kc)*TC + t] = hs[c*TC + t, kc*128 + p], so each chunk's
    # DMA is 128 contiguous 8 KB lines instead of 1024 separate 1 KB lines
    hsR = nc.dram_tensor("hsR", [128, KC * T], f16, kind="ExternalInput").ap()
    gupg = nc.dram_tensor("gupg", [128, NJ * KC * 128], f16, kind="ExternalInput").ap()
    gupu = nc.dram_tensor("gupu", [128, NJ * KC * 128], f16, kind="ExternalInput").ap()
    # misc packs [gb | ub | wt] to save DMA-issue slots (~0.6us each)
    misc = nc.dram_tensor("misc", [128, 2 * NJ + T // 128], f32, kind="ExternalInput").ap()
    dwT = nc.dram_tensor("dwT", [128, KC * H], f16, kind="ExternalInput").ap()
    out = nc.dram_tensor("out", [T, H], f32, kind="ExternalOutput").ap()

    with tile.TileContext(nc) as tc_:
        with tc_.tile_pool(name="wpool", bufs=1) as wpool, \
             tc_.tile_pool(name="hpool", bufs=2) as hpool, \
             tc_.tile_pool(name="apool", bufs=2) as apool, \
             tc_.tile_pool(name="spool", bufs=8) as spool, \
             tc_.tile_pool(name="opool", bufs=3) as opool, \
             tc_.tile_pool(name="ps0", bufs=1, space="PSUM") as ps0, \
             tc_.tile_pool(name="ps1", bufs=2, space="PSUM") as ps1, \
             tc_.tile_pool(name="ps2", bufs=3, space="PSUM") as ps2:

            gupg_r = wpool.tile([128, NJ * KC * 128], f16)
            gupu_r = wpool.tile([128, NJ * KC * 128], f16)
            dwT_r = wpool.tile([128, KC * H], f16)
            misc_r = wpool.tile([128, 2 * NJ + T // 128], f32)
            gb_r = misc_r[:, 0:NJ]
            ub_r = misc_r[:, NJ:2*NJ]
            w_r = misc_r[:, 2*NJ:]
            dummy = wpool.tile([128, 512], f16)
            pdum = ps0.tile([128, 512], f32)

            # DMA order matches consumption order: the chunk-0 tokens, the
            # bias/route pack, then gate j-blocks (the gate pass consumes them
            # in this order), chunk-1 tokens, the up half, down weights, and
            # the chunk-2 prefetch.
            JB = KC * 128   # columns per (j) block of gup
            # chunk-0 tokens as ONE transfer: starting the gate pass before
            # the DMA pipe is fully ramped leaves >40% PE-idle in the paced
            # matmuls, which re-throttles the HAM clock gate mid-chunk; the
            # warm-up dummies below keep the PE continuously busy instead and
            # the real matmuls then run gap-free.
            hs0 = hpool.tile([128, KC * CHUNKS[0]], f16, tag="hs")
            nc.sync.dma_start(hs0[:], hsR[:, 0:KC*TC])
            nc.sync.dma_start(misc_r[:], misc[:])
            for j in range(NJ):
                nc.sync.dma_start(gupg_r[:, j*JB:(j+1)*JB], gupg[:, j*JB:(j+1)*JB])
            hs1 = hpool.tile([128, KC * CHUNKS[1]], f16, tag="hs")
            nc.sync.dma_start(hs1[:], hsR[:, KC*TC:2*KC*TC])
            for m in range(4):
                nc.sync.dma_start(gupu_r[:, 2*m*JB:2*(m+1)*JB], gupu[:, 2*m*JB:2*(m+1)*JB])
            nc.sync.dma_start(dwT_r[:, :KC*H//2], dwT[:, :KC*H//2])
            nc.sync.dma_start(dwT_r[:, KC*H//2:], dwT[:, KC*H//2:])

            # PE warm-up: dependency-free matmuls fill the HAM activity window
            # (3.4us) during the DMA preamble so the real matmuls start at
            # 2.4 GHz instead of 1.2 GHz. 16 of them (~4.7us from ~8-10us)
            # bridge to the ~14us arrival of the first chunk's data without
            # re-throttling in between.
            nc.vector.memset(dummy[:], 0)
            for i in range(16):
                nc.tensor.matmul(pdum[:], dummy[:, 0:128], dummy[:],
                                 start=(i == 0), stop=(i == 15))

            t_off = 0
            for c, TCc in enumerate(CHUNKS):
                if c == 0:
                    hs_r = hs0
                elif c == 1:
                    hs_r = hs1
                else:
                    hs_r = hpool.tile([128, KC * TCc], f16, tag="hs")
                    nc.sync.dma_start(hs_r[:], hsR[:, c*KC*TC:(c+1)*KC*TC])

                act_r = apool.tile([128, NJ * TCc], f16, tag="act")
                s2s = []
                for j in range(NJ):     # gate pass
                    pg = ps1.tile([128, TCc], f32, tag="pg")
                    for kc in range(KC):
                        nc.tensor.matmul(pg[:], gupg_r[:, (j*KC + kc)*128:(j*KC + kc + 1)*128],
                                         hs_r[:, kc*TCc:(kc+1)*TCc],
                                         start=(kc == 0), stop=(kc == KC - 1))
                    s2 = spool.tile([128, TCc], f32, tag="s2")
                    nc.scalar.activation(s2[:], pg[:], AF.Silu,
                                         bias=gb_r[:, j:j+1], scale=1.702)
                    s2s.append(s2)
                for j in range(NJ):     # up pass: act = (up + ub + 1) * silu_out
                    pu = ps1.tile([128, TCc], f32, tag="pu")
                    for kc in range(KC):
                        nc.tensor.matmul(pu[:], gupu_r[:, (j*KC + kc)*128:(j*KC + kc + 1)*128],
                                         hs_r[:, kc*TCc:(kc+1)*TCc],
                                         start=(kc == 0), stop=(kc == KC - 1))
                    nc.vector.scalar_tensor_tensor(act_r[:, j*TCc:(j+1)*TCc], pu[:],
                                                   ub_r[:, j:j+1], s2s[j][:],
                                                   op0=ALU.add, op1=ALU.mult)

                for tt in range(TCc // 128):
                    gt = (t_off // 128) + tt
                    wcol = w_r[:, gt:gt+1]
                    ot = opool.tile([128, H], f32, tag="ot")
                    for hh in range(H // 512):
                        p2 = ps2.tile([128, 512], f32, tag="p2")
                        for ic in range(KC):
                            nc.tensor.matmul(p2[:], act_r[:, ic*TCc + tt*128:ic*TCc + (tt+1)*128],
                                             dwT_r[:, ic*H + hh*512:ic*H + (hh+1)*512],
                                             start=(ic == 0), stop=(ic == KC - 1))
                        # per-half DMA so the final transfer trails the last
                        # matmul by ~1.5us instead of ~4us; the very last tile
                        # goes in two 256-col pieces to shorten the tail chain
                        last_tile = (c == len(CHUNKS) - 1 and tt == TCc // 128 - 1
                                     and hh == H // 512 - 1)
                        for a, b in ([(0, 256), (256, 512)] if last_tile else [(0, 512)]):
                            nc.vector.tensor_scalar_mul(
                                ot[:, hh*512 + a:hh*512 + b], p2[:, a:b], wcol)
                            nc.sync.dma_start(
                                out[t_off + tt*128:t_off + (tt+1)*128, hh*512 + a:hh*512 + b],
                                ot[:, hh*512 + a:hh*512 + b])
                t_off += TCc
    nc.compile()
    return nc


def _get_nc():
    if 'nc' not in _CACHE:
        _CACHE['nc'] = _build()
    return _CACHE['nc']


def _make_in_maps(hidden_states, routing_weights, gate_up_proj, gate_up_proj_bias,
                  down_proj, down_proj_bias):
    hs = np.asarray(hidden_states, dtype=np.float32)
    rw = np.asarray(routing_weights, dtype=np.float32)
    gupw = np.asarray(gate_up_proj, dtype=np.float32)
    gupb = np.asarray(gate_up_proj_bias, dtype=np.float32)
    dw = np.asarray(down_proj, dtype=np.float32)
    NCHUNK = T // TC
    hsR = np.ascontiguousarray(
        hs.T.astype(np.float16).reshape(KC, 128, NCHUNK, TC)
        .transpose(1, 2, 0, 3).reshape(128, KC * T))
    in_maps = []
    for e in range(N_CORES):
        g = gupw[e]
        # (j, kc)-block layout: col block j*KC+kc holds gate[kc*128:(kc+1)*128,
        # j*128:(j+1)*128], so the j-loop consumes weights in DMA order.
        gate = g[:, 0::2].astype(np.float16)
        up = g[:, 1::2].astype(np.float16)
        gate_b = gate.reshape(KC, 128, NJ, 128).transpose(1, 2, 0, 3).reshape(128, NJ*KC*128)
        up_b = up.reshape(KC, 128, NJ, 128).transpose(1, 2, 0, 3).reshape(128, NJ*KC*128)
        # silu(1.702*(x + b)) = silu(1.702*x + 1.702*b); the 1/1.702 glu
        # scale is folded into dwT below.
        dwTe = (dw[e].T / np.float32(1.702)).astype(np.float16)
        misc = np.concatenate([
            (1.702 * gupb[e, 0::2]).reshape(NJ, 128).T,
            (gupb[e, 1::2] + 1.0).reshape(NJ, 128).T,
            rw[:, e].reshape(T // 128, 128).T,
        ], axis=1).astype(np.float32)
        in_maps.append({
            "hsR": hsR,
            "gupg": np.ascontiguousarray(gate_b),
            "gupu": np.ascontiguousarray(up_b),
            "misc": np.ascontiguousarray(misc),
            "dwT": np.ascontiguousarray(dwTe.reshape(KC, 128, H).transpose(1, 0, 2).reshape(128, KC*H)),
        })
    return in_maps


def _assemble(results, routing_weights, down_proj_bias):
    # Sum the 8 expert partials and add the routing-weighted down-bias term
    # (both in fp32 on the host).
    rw = np.asarray(routing_weights, dtype=np.float32)
    db = np.asarray(down_proj_bias, dtype=np.float32)
    out = rw @ db  # [T, E] @ [E, H]
    for r in range(N_CORES):
        out += results[r]["out"]
    return out


def kernel(hidden_states, routing_weights, gate_up_proj, gate_up_proj_bias,
           down_proj, down_proj_bias):
    from concourse import bass_utils
    in_maps = _make_in_maps(hidden_states, routing_weights, gate_up_proj,
                            gate_up_proj_bias, down_proj, down_proj_bias)
    nc = _get_nc()
    try:
        res = bass_utils.run_bass_kernel_spmd(nc, in_maps, core_ids=list(range(N_CORES)))
    except Exception:
        # One retry in case a previous process left a core wedged.
        res = bass_utils.run_bass_kernel_spmd(nc, in_maps, core_ids=list(range(N_CORES)))
    return _assemble(res.results, routing_weights, down_proj_bias)


# revision 31
# speedup vs baseline: 1.0010x; 1.0010x over previous
"""MoE dense all-experts (GPT-OSS Experts forward) on 8 Trainium2 NeuronCores.

Expert-parallel sharding: core e holds expert e's weights and computes its
weighted contribution

    partial_e[t, h] = w[t, e] * ((up + 1) * silu(1.702 * gate) @ down_e.T)

with [gate | up] = hs @ gup_e + bias (the host de-interleaves gup's even/odd
columns so gate/up become contiguous halves). Each core writes its full
[T, H] partial to DRAM; the host sums the 8 partials and adds the
routing-weighted down-bias term (routing_weights @ down_bias) in fp32.

All matmul operands are float16 (fp32 PSUM accumulation): fp32/fp32r moving
operands stream through the PE at ~0.56 ns/col while 16-bit operands stream
at 1 col/cycle @ 2.4 GHz (0.417 ns/col), a 1.35x speedup at identical MAC
count; fp16's 10-bit mantissa keeps the end-to-end relative error ~5e-4.
Weights are staged in (j, kc)-block order so the gate pass starts after
~1.25 MB of DMA instead of waiting for the full weight tensor, and 16
dependency-free warm-up matmuls keep the PE busy through the DMA preamble
so the HAM clock gate releases (1.2 -> 2.4 GHz) before real work starts.

Stage 1 computes [f, t] tiles (gate pass feeding the ScalarE Silu LUT, then
up pass fused with the silu output via scalar_tensor_tensor into fp16
act[i, t]); stage 2 computes out[t, h] = (act.T @ dwT) * w[t] with act as
the stationary operand and a single VectorE tensor_scalar epilogue.
"""
import sys
if '/opt/trn_rl_repo' not in sys.path:
    sys.path.insert(0, '/opt/trn_rl_repo')
import numpy as np

E, H, I, T = 8, 1024, 1024, 4096
N_CORES = 8
TC = 512               # token chunk
CHUNKS = [TC] * 8
KC = H // 128          # contraction chunks (H == I == 1024)
NJ = I // 128          # gate/up row tiles

_CACHE = {}


def _build():
    import concourse.bacc as bacc
    import concourse.tile as tile
    import concourse.mybir as mybir
    f32 = mybir.dt.float32
    f16 = mybir.dt.float16
    AF = mybir.ActivationFunctionType
    ALU = mybir.AluOpType

    nc = bacc.Bacc("TRN2", target_bir_lowering=False, debug=False,
                   enable_asserts=False, num_devices=N_CORES)
    # hsR is host-pre-arranged into the exact SBUF chunk layout
    # hsR[p, (c*KC + kc)*TC + t] = hs[c*TC + t, kc*128 + p], so each chunk's
    # DMA is 128 contiguous 8 KB lines instead of 1024 separate 1 KB lines
    hsR = nc.dram_tensor("hsR", [128, KC * T], f16, kind="ExternalInput").ap()
    gupg = nc.dram_tensor("gupg", [128, NJ * KC * 128], f16, kind="ExternalInput").ap()
    gupu = nc.dram_tensor("gupu", [128, NJ * KC * 128], f16, kind="ExternalInput").ap()
    # misc packs [gb | ub | wt] to save DMA-issue slots (~0.6us each)
    misc = nc.dram_tensor("misc", [128, 2 * NJ + T // 128], f32, kind="ExternalInput").ap()
    dwT = nc.dram_tensor("dwT", [128, KC * H], f16, kind="ExternalInput").ap()
    # partials ship as fp16 (host sums in fp32): halves the out-DMA traffic
    # that pins the exec-window close, costs ~3e-4 added relative error
    out = nc.dram_tensor("out", [T, H], f16, kind="ExternalOutput").ap()

    with tile.TileContext(nc) as tc_:
        with tc_.tile_pool(name="wpool", bufs=1) as wpool, \
             tc_.tile_pool(name="hpool", bufs=2) as hpool, \
             tc_.tile_pool(name="apool", bufs=2) as apool, \
             tc_.tile_pool(name="spool", bufs=8) as spool, \
             tc_.tile_pool(name="opool", bufs=3) as opool, \
             tc_.tile_pool(name="ps0", bufs=1, space="PSUM") as ps0, \
             tc_.tile_pool(name="ps1", bufs=2, space="PSUM") as ps1, \
             tc_.tile_pool(name="ps2", bufs=3, space="PSUM") as ps2:

            gupg_r = wpool.tile([128, NJ * KC * 128], f16)
            gupu_r = wpool.tile([128, NJ * KC * 128], f16)
            dwT_r = wpool.tile([128, KC * H], f16)
            misc_r = wpool.tile([128, 2 * NJ + T // 128], f32)
            gb_r = misc_r[:, 0:NJ]
            ub_r = misc_r[:, NJ:2*NJ]
            w_r = misc_r[:, 2*NJ:]
            dummy = wpool.tile([128, 512], f16)
            pdum = ps0.tile([128, 512], f32)

            # DMA order matches consumption order: the chunk-0 tokens, the
            # bias/route pack, then gate j-blocks (the gate pass consumes them
            # in this order), chunk-1 tokens, the up half, down weights, and
            # the chunk-2 prefetch.
            JB = KC * 128   # columns per (j) block of gup
            # chunk-0 tokens as ONE transfer: starting the gate pass before
            # the DMA pipe is fully ramped leaves >40% PE-idle in the paced
            # matmuls, which re-throttles the HAM clock gate mid-chunk; the
            # warm-up dummies below keep the PE continuously busy instead and
            # the real matmuls then run gap-free.
            hs0 = hpool.tile([128, KC * CHUNKS[0]], f16, tag="hs")
            nc.sync.dma_start(hs0[:], hsR[:, 0:KC*TC])
            nc.sync.dma_start(misc_r[:], misc[:])
            for j in range(NJ):
                nc.sync.dma_start(gupg_r[:, j*JB:(j+1)*JB], gupg[:, j*JB:(j+1)*JB])
            hs1 = hpool.tile([128, KC * CHUNKS[1]], f16, tag="hs")
            nc.sync.dma_start(hs1[:], hsR[:, KC*TC:2*KC*TC])
            for m in range(4):
                nc.sync.dma_start(gupu_r[:, 2*m*JB:2*(m+1)*JB], gupu[:, 2*m*JB:2*(m+1)*JB])
            nc.sync.dma_start(dwT_r[:, :KC*H//2], dwT[:, :KC*H//2])
            nc.sync.dma_start(dwT_r[:, KC*H//2:], dwT[:, KC*H//2:])

            # PE warm-up: dependency-free matmuls fill the HAM activity window
            # (3.4us) during the DMA preamble so the real matmuls start at
            # 2.4 GHz instead of 1.2 GHz. 16 of them (~4.7us from ~8-10us)
            # bridge to the ~14us arrival of the first chunk's data without
            # re-throttling in between.
            nc.vector.memset(dummy[:], 0)
            for i in range(16):
                nc.tensor.matmul(pdum[:], dummy[:, 0:128], dummy[:],
                                 start=(i == 0), stop=(i == 15))

            t_off = 0
            for c, TCc in enumerate(CHUNKS):
                if c == 0:
                    hs_r = hs0
                elif c == 1:
                    hs_r = hs1
                else:
                    hs_r = hpool.tile([128, KC * TCc], f16, tag="hs")
                    nc.sync.dma_start(hs_r[:], hsR[:, c*KC*TC:(c+1)*KC*TC])

                act_r = apool.tile([128, NJ * TCc], f16, tag="act")
                s2s = []
                for j in range(NJ):     # gate pass
                    pg = ps1.tile([128, TCc], f32, tag="pg")
                    for kc in range(KC):
                        nc.tensor.matmul(pg[:], gupg_r[:, (j*KC + kc)*128:(j*KC + kc + 1)*128],
                                         hs_r[:, kc*TCc:(kc+1)*TCc],
                                         start=(kc == 0), stop=(kc == KC - 1))
                    s2 = spool.tile([128, TCc], f32, tag="s2")
                    nc.scalar.activation(s2[:], pg[:], AF.Silu,
                                         bias=gb_r[:, j:j+1], scale=1.702)
                    s2s.append(s2)
                for j in range(NJ):     # up pass: act = (up + ub + 1) * silu_out
                    pu = ps1.tile([128, TCc], f32, tag="pu")
                    for kc in range(KC):
                        nc.tensor.matmul(pu[:], gupu_r[:, (j*KC + kc)*128:(j*KC + kc + 1)*128],
                                         hs_r[:, kc*TCc:(kc+1)*TCc],
                                         start=(kc == 0), stop=(kc == KC - 1))
                    nc.vector.scalar_tensor_tensor(act_r[:, j*TCc:(j+1)*TCc], pu[:],
                                                   ub_r[:, j:j+1], s2s[j][:],
                                                   op0=ALU.add, op1=ALU.mult)

                for tt in range(TCc // 128):
                    gt = (t_off // 128) + tt
                    wcol = w_r[:, gt:gt+1]
                    ot = opool.tile([128, H], f16, tag="ot")
                    for hh in range(H // 512):
                        p2 = ps2.tile([128, 512], f32, tag="p2")
                        for ic in range(KC):
                            nc.tensor.matmul(p2[:], act_r[:, ic*TCc + tt*128:ic*TCc + (tt+1)*128],
                                             dwT_r[:, ic*H + hh*512:ic*H + (hh+1)*512],
                                             start=(ic == 0), stop=(ic == KC - 1))
                        # per-half DMA so the final transfer trails the last
                        # matmul by ~1.5us instead of ~4us; the very last tile
                        # goes in two 256-col pieces to shorten the tail chain
                        last_tile = (c == len(CHUNKS) - 1 and tt == TCc // 128 - 1
                                     and hh == H // 512 - 1)
                        for a, b in ([(0, 256), (256, 512)] if last_tile else [(0, 512)]):
                            nc.vector.tensor_scalar_mul(
                                ot[:, hh*512 + a:hh*512 + b], p2[:, a:b], wcol)
                            nc.sync.dma_start(
                                out[t_off + tt*128:t_off + (tt+1)*128, hh*512 + a:hh*512 + b],
                                ot[:, hh*512 + a:hh*512 + b])
                t_off += TCc
    nc.compile()
    return nc


def _get_nc():
    if 'nc' not in _CACHE:
        _CACHE['nc'] = _build()
    return _CACHE['nc']


def _make_in_maps(hidden_states, routing_weights, gate_up_proj, gate_up_proj_bias,
                  down_proj, down_proj_bias):
    hs = np.asarray(hidden_states, dtype=np.float32)
    rw = np.asarray(routing_weights, dtype=np.float32)
    gupw = np.asarray(gate_up_proj, dtype=np.float32)
    gupb = np.asarray(gate_up_proj_bias, dtype=np.float32)
    dw = np.asarray(down_proj, dtype=np.float32)
    NCHUNK = T // TC
    hsR = np.ascontiguousarray(
        hs.T.astype(np.float16).reshape(KC, 128, NCHUNK, TC)
        .transpose(1, 2, 0, 3).reshape(128, KC * T))
    in_maps = []
    for e in range(N_CORES):
        g = gupw[e]
        # (j, kc)-block layout: col block j*KC+kc holds gate[kc*128:(kc+1)*128,
        # j*128:(j+1)*128], so the j-loop consumes weights in DMA order.
        gate = g[:, 0::2].astype(np.float16)
        up = g[:, 1::2].astype(np.float16)
        gate_b = gate.reshape(KC, 128, NJ, 128).transpose(1, 2, 0, 3).reshape(128, NJ*KC*128)
        up_b = up.reshape(KC, 128, NJ, 128).transpose(1, 2, 0, 3).reshape(128, NJ*KC*128)
        # silu(1.702*(x + b)) = silu(1.702*x + 1.702*b); the 1/1.702 glu
        # scale is folded into dwT below.
        dwTe = (dw[e].T / np.float32(1.702)).astype(np.float16)
        misc = np.concatenate([
            (1.702 * gupb[e, 0::2]).reshape(NJ, 128).T,
            (gupb[e, 1::2] + 1.0).reshape(NJ, 128).T,
            rw[:, e].reshape(T // 128, 128).T,
        ], axis=1).astype(np.float32)
        in_maps.append({
            "hsR": hsR,
            "gupg": np.ascontiguousarray(gate_b),
            "gupu": np.ascontiguousarray(up_b),
            "misc": np.ascontiguousarray(misc),
            "dwT": np.ascontiguousarray(dwTe.reshape(KC, 128, H).transpose(1, 0, 2).reshape(128, KC*H)),
        })
    return in_maps


def _assemble(results, routing_weights, down_proj_bias):
    # Sum the 8 expert partials and add the routing-weighted down-bias term
    # (both in fp32 on the host).
    rw = np.asarray(routing_weights, dtype=np.float32)
    db = np.asarray(down_proj_bias, dtype=np.float32)
    out = rw @ db  # [T, E] @ [E, H]
    for r in range(N_CORES):
        out += results[r]["out"].astype(np.float32)
    return out


def kernel(hidden_states, routing_weights, gate_up_proj, gate_up_proj_bias,
           down_proj, down_proj_bias):
    from concourse import bass_utils
    in_maps = _make_in_maps(hidden_states, routing_weights, gate_up_proj,
                            gate_up_proj_bias, down_proj, down_proj_bias)
    nc = _get_nc()
    try:
        res = bass_utils.run_bass_kernel_spmd(nc, in_maps, core_ids=list(range(N_CORES)))
    except Exception:
        # One retry in case a previous process left a core wedged.
        res = bass_utils.run_bass_kernel_spmd(nc, in_maps, core_ids=list(range(N_CORES)))
    return _assemble(res.results, routing_weights, down_proj_bias)
